# revision 26
# baseline (speedup 1.0000x reference)
"""AttentionSinkPrefill Trainium2 kernel v3 (8 NeuronCores, sequence-parallel).

Module:   Y = AttnSinkPrefill(X) with sink=4, window=256, causal GQA
          (16 q heads, 4 kv heads, head_dim 64, d_model 1024, B=2, T=2048).

Sharding: sequence-parallel over T.  Core c handles queries
          [256c, 256c+256) for both batches; it needs X rows
          [256c-256, 256c+256) (zero-padded at the left edge) plus the 4
          sink rows, computes its o_proj output rows completely -- no
          collectives, outputs concatenate on host.

v3 (178.9us -> target ~140us):
  - X^T is laid out on the host (pure data layout, like the weight cast)
    so the on-device transpose stage disappears; projections start as
    soon as the first X^T tiles land
  - heads paired WITHIN a kv group (identity head order); K^T rows are
    duplicated onto both partition halves so both heads of a pair read
    the same V tiles -- the pair's two Y chains merge into 5 matmuls
    with [128, 2, 256] strided APs (was 10)
  - attention scores/exp/masks per head unchanged; softmax denominator:
    one ACT row copy -> K=1 f32r broadcast matmul -> fast reciprocal
    (v2.1 chain) per pair
"""

import os
import sys
from contextlib import ExitStack

import numpy as np

# engine-choice switches for bisecting hardware issues
K_SINKMUL = os.environ.get("K_SINKMUL", "dve")   # dve | gpsimd
K_MEMSET = os.environ.get("K_MEMSET", "dve")     # dve | gpsimd

sys.path.insert(0, "/opt/trn_rl_repo")

import concourse.bass as bass
import concourse.bacc as bacc
import concourse.mybir as mybir
import concourse.tile as tile
from concourse.bass_utils import run_bass_kernel_spmd

# ---------------------------------------------------------------- constants
D = 1024          # d_model
NH = 16           # q heads
NKV = 4           # kv heads
HD = 64           # head dim
SINK = 4          # attention sink width
WIN = 256         # sliding window
B = 2
T = 2048
NCORES = 8
QB = T // NCORES  # queries per core = 256
KW = 2 * QB       # window key rows per core = 512
KCOL = KW + SINK  # X^T key columns per batch = 516

F32 = mybir.dt.float32
FR = mybir.dt.float32r
BF = mybir.dt.bfloat16

AF = mybir.ActivationFunctionType


# ================================================================ program
def build_nc():
    nc = bacc.Bacc()

    # inputs packed into few large blobs: descriptor setup for each
    # dma_start serializes on the sync engine, so fewer/bigger transfers
    # start the pipeline sooner
    xt_d = nc.dram_tensor("XT", [128, 8 * B * KCOL], BF, kind="ExternalInput")
    wkv_d = nc.dram_tensor("WKV", [128, 8 * 512], BF, kind="ExternalInput")
    wq_d = nc.dram_tensor("WQA", [128, 8 * 1024], BF, kind="ExternalInput")
    wo_d = nc.dram_tensor("WOA", [128, 8 * 1024], BF, kind="ExternalInput")
    msk_d = nc.dram_tensor("MSK", [128, 896], BF, kind="ExternalInput")
    oner_d = nc.dram_tensor("ONER", [65, 64], FR, kind="ExternalInput")
    out_d = nc.dram_tensor("out", [B, QB, D], BF, kind="ExternalOutput")

    with nc.allow_low_precision(reason="bf16 matmul operands"), \
            tile.TileContext(nc) as tc, ExitStack() as ctx:
        consts = ctx.enter_context(tc.tile_pool(name="consts", bufs=1))
        wpool = ctx.enter_context(tc.tile_pool(name="wpool", bufs=1))
        big = ctx.enter_context(tc.tile_pool(name="big", bufs=1))
        xtp = ctx.enter_context(tc.tile_pool(name="xt", bufs=1))
        qkv = ctx.enter_context(tc.tile_pool(name="qkv", bufs=1))
        ppool = ctx.enter_context(tc.tile_pool(name="pp", bufs=3))
        ypool = ctx.enter_context(tc.tile_pool(name="yp", bufs=1))
        spool = ctx.enter_context(tc.tile_pool(name="sp", bufs=2))
        opool = ctx.enter_context(tc.tile_pool(name="op", bufs=2))
        # PSUM: tag "s" 2x[128,1024]f32 = 4 banks, tag "blk" 2x1 = 2,
        # tag "ys" 2x[65,512] = 2  -> exactly 8 banks
        psS = ctx.enter_context(tc.tile_pool(name="psS", bufs=2, space="PSUM"))
        psB = ctx.enter_context(tc.tile_pool(name="psB", bufs=2, space="PSUM"))
        psY = ctx.enter_context(tc.tile_pool(name="psY", bufs=2, space="PSUM"))

        # -------- DMA: X^T first (projections start on it), then K/V
        # weights (small, consumed first), masks, Q weights; Wo is issued
        # later (only needed at o_proj)
        xall = xtp.tile([128, 8 * B * KCOL], BF, tag="xall", name="xall")
        nc.sync.dma_start(xall[:], xt_d[:])
        xt = [xall[:, d * B * KCOL:(d + 1) * B * KCOL] for d in range(8)]

        wkv = wpool.tile([128, 8 * 512], BF, tag="wkv", name="wkv")
        nc.sync.dma_start(wkv[:], wkv_d[:])
        wk = [wkv[:, d * 512:d * 512 + 256] for d in range(8)]
        wv = [wkv[:, d * 512 + 256:(d + 1) * 512] for d in range(8)]

        mall = consts.tile([128, 896], BF, tag="mall")
        nc.sync.dma_start(mall[:], msk_d[:])
        m1 = mall[:, 0:384]
        m2 = mall[:, 384:640]
        ms = mall[0:SINK, 640:896]
        ones = consts.tile([65, 64], FR, tag="ones")
        nc.sync.dma_start(ones[:], oner_d[:])

        wqa = wpool.tile([128, 8 * 1024], BF, tag="wqa", name="wqa")
        nc.sync.dma_start(wqa[:], wq_d[:])
        wq = [wqa[:, d * 1024:(d + 1) * 1024] for d in range(8)]

        mseng = nc.gpsimd if K_MEMSET == "gpsimd" else nc.vector
        # -------- persistent per-core tensors
        qt = [qkv.tile([128, B * QB], BF, tag=f"qt{m}", name=f"qt{m}")
              for m in range(8)]
        # K^T per kv head, duplicated onto both partition halves so both
        # heads of a same-group pair can use their own 64-partition slice
        ktd = [qkv.tile([128, B * KW], BF, tag=f"ktd{g}", name=f"ktd{g}")
               for g in range(NKV)]
        # zero-padded duplicated sink K^T: [128, key 0:4 real | 4:128 zero]
        ktp = {}
        for g in range(NKV):
            for b in range(B):
                tl = qkv.tile([128, 128], BF, tag=f"ktp{g}{b}", name=f"ktp{g}{b}")
                mseng.memset(tl[:], 0.0)
                ktp[(g, b)] = tl
        # V keys-major with a ones column per kv head (softmax denominator)
        vt = {}
        for tki in range(4):
            for b in range(B):
                tl = qkv.tile([128, NKV * 65], BF,
                              tag=f"vt{tki}{b}", name=f"vt{tki}{b}")
                mseng.memset(tl[:, 64:NKV * 65:65], 1.0)
                vt[(tki, b)] = tl
        vs = {}
        for b in range(B):
            tl = qkv.tile([SINK, NKV * 65], BF, tag=f"vs{b}", name=f"vs{b}")
            mseng.memset(tl[0:SINK, 64:NKV * 65:65], 1.0)
            vs[b] = tl
        yt = [ypool.tile([128, B * QB], BF, tag=f"yt{m}", name=f"yt{m}")
              for m in range(8)]

        # ---------------- stage 2: projections.  Emission order matters
        # (engine queues execute in order): K(b0), V(b0) first -- their
        # weights are small and land early while X^T still streams -- then
        # Q (bigger Wq lands meanwhile), then K(b1), V(b1).
        def q_proj():
            for m in range(8):
                ps = psB.tile([128, 512], F32, tag="blk", name=f"qps{m}")
                for d in range(8):
                    rhs = xt[d].rearrange(
                        "p (b c) -> p b c", b=B
                    )[:, :, KW - QB:KW]
                    nc.tensor.matmul(
                        ps[:],
                        wq[d][:, m * 128:(m + 1) * 128],
                        rhs,
                        start=(d == 0), stop=(d == 7),
                    )
                nc.scalar.copy(qt[m][:], ps[:])

        def kv_proj(b):
            for m in range(2):
                gA, gB = 2 * m, 2 * m + 1
                ps = psB.tile([128, 512], F32, tag="blk", name=f"kps{b}{m}")
                for d in range(8):
                    nc.tensor.matmul(
                        ps[:],
                        wk[d][:, m * 128:(m + 1) * 128],
                        xt[d][:, b * KCOL:b * KCOL + KW],
                        start=(d == 0), stop=(d == 7),
                    )
                # top half of each dup tile via engine copy (no partition
                # shift), the other half via SBUF->SBUF DMA duplication
                nc.vector.tensor_copy(
                    ktd[gA][0:64, b * KW:(b + 1) * KW], ps[0:64, :])
                nc.vector.tensor_copy(
                    ktd[gB][64:128, b * KW:(b + 1) * KW], ps[64:128, :])
                nc.sync.dma_start(
                    ktd[gA][64:128, b * KW:(b + 1) * KW],
                    ktd[gA][0:64, b * KW:(b + 1) * KW])
                nc.sync.dma_start(
                    ktd[gB][0:64, b * KW:(b + 1) * KW],
                    ktd[gB][64:128, b * KW:(b + 1) * KW])

                ps2 = psB.tile([128, 512], F32, tag="blk", name=f"ksps{b}{m}")
                for d in range(8):
                    nc.tensor.matmul(
                        ps2[:, 0:SINK],
                        wk[d][:, m * 128:(m + 1) * 128],
                        xt[d][:, b * KCOL + KW:b * KCOL + KCOL],
                        start=(d == 0), stop=(d == 7),
                    )
                nc.vector.tensor_copy(
                    ktp[(gA, b)][0:64, 0:SINK], ps2[0:64, 0:SINK])
                nc.vector.tensor_copy(
                    ktp[(gB, b)][64:128, 0:SINK], ps2[64:128, 0:SINK])
                nc.sync.dma_start(
                    ktp[(gA, b)][64:128, 0:SINK], ktp[(gA, b)][0:64, 0:SINK])
                nc.sync.dma_start(
                    ktp[(gB, b)][0:64, 0:SINK], ktp[(gB, b)][64:128, 0:SINK])

            for tki in range(4):
                ps = psB.tile([128, 512], F32, tag="blk", name=f"vps{b}{tki}")
                for d in range(8):
                    nc.tensor.matmul(
                        ps[:, 0:NKV * HD],
                        xt[d][:, b * KCOL + tki * 128:b * KCOL + (tki + 1) * 128],
                        wv[d][:],
                        start=(d == 0), stop=(d == 7),
                    )
                nc.vector.tensor_copy(
                    vt[(tki, b)][:].rearrange("p (g c) -> p g c", c=65)[:, :, 0:HD],
                    ps[:, 0:NKV * HD].rearrange("p (g c) -> p g c", c=HD),
                )
            ps = psB.tile([128, 512], F32, tag="blk", name=f"vsps{b}")
            for d in range(8):
                nc.tensor.matmul(
                    ps[0:SINK, 0:NKV * HD],
                    xt[d][:, b * KCOL + KW:b * KCOL + KCOL],
                    wv[d][:],
                    start=(d == 0), stop=(d == 7),
                )
            nc.vector.tensor_copy(
                vs[b][0:SINK, :].rearrange("p (g c) -> p g c", c=65)[:, :, 0:HD],
                ps[0:SINK, 0:NKV * HD].rearrange("p (g c) -> p g c", c=HD),
            )

        kv_proj(0)
        kv_proj(1)
        q_proj()

        # Wo DMA now: streams during attention, consumed by o_proj
        woa = big.tile([128, 8 * 1024], BF, tag="woa", name="woa")
        nc.sync.dma_start(woa[:], wo_d[:])
        wo = [woa[:, m * 1024:(m + 1) * 1024] for m in range(8)]

        # ---------------- stage 3: attention
        # p layout per head (1024 bf16 cols inside the pair tile):
        #   [0:256]    sink scores (keys 0:4 real via zero-padded ktp)
        #   [256:512]  key tile T(-1) = window cols 128:256, queries 0:256
        #   [512:768]  key tile T(0)  = window cols 256:384, queries 0:256
        #   [768:896]  key tile T(-2) = window cols 0:128,   queries 0:128
        #   [896:1024] key tile T(1)  = window cols 384:512, queries 128:256
        def scores_half(b, pr, kb, p, off):
            """scores + exp + masks for one head (partition base kb)."""
            g = pr // 2
            qall = qt[pr][kb:kb + 64, b * QB:(b + 1) * QB]
            qhb0 = qt[pr][kb:kb + 64, b * QB:b * QB + 128]
            qhb1 = qt[pr][kb:kb + 64, b * QB + 128:(b + 1) * QB]
            kw0 = b * KW

            sp = psS.tile([128, 1024], F32, tag="s", name=f"s{b}{pr}{kb}")
            nc.tensor.matmul(sp[:, 0:256], ktp[(g, b)][kb:kb + 64, :],
                             qall, start=True, stop=True)
            nc.tensor.matmul(sp[:, 256:512],
                             ktd[g][kb:kb + 64, kw0 + 128:kw0 + 256],
                             qall, start=True, stop=True)
            nc.tensor.matmul(sp[:, 512:768],
                             ktd[g][kb:kb + 64, kw0 + 256:kw0 + 384],
                             qall, start=True, stop=True)
            nc.tensor.matmul(sp[:, 768:896],
                             ktd[g][kb:kb + 64, kw0 + 0:kw0 + 128],
                             qhb0, start=True, stop=True)
            nc.tensor.matmul(sp[:, 896:1024],
                             ktd[g][kb:kb + 64, kw0 + 384:kw0 + 512],
                             qhb1, start=True, stop=True)

            nc.scalar.activation(p[:, off:off + 1024], sp[:], AF.Exp)
            nc.vector.tensor_mul(p[:, off + 256:off + 640],
                                 p[:, off + 256:off + 640], m1)
            nc.vector.tensor_mul(p[:, off + 768:off + 1024],
                                 p[:, off + 768:off + 1024], m2)
            smeng = nc.gpsimd if K_SINKMUL == "gpsimd" else nc.vector
            smeng.tensor_mul(p[0:SINK, off:off + 256],
                             p[0:SINK, off:off + 256], ms)

        def o_proj(b):
            for mq2 in range(2):
                for nk in range(2):
                    po = psB.tile([128, 512], F32, tag="blk",
                                  name=f"po{b}{mq2}{nk}")
                    for m in range(8):
                        nc.tensor.matmul(
                            po[:],
                            yt[m][:, b * QB + mq2 * 128:b * QB + (mq2 + 1) * 128],
                            wo[m][:, nk * 512:(nk + 1) * 512],
                            start=(m == 0), stop=(m == 7),
                        )
                    ost = opool.tile([128, 512], BF, tag="ost",
                                     name=f"o{b}{mq2}{nk}")
                    nc.vector.tensor_copy(ost[:], po[:])
                    nc.sync.dma_start(
                        out_d[b, mq2 * 128:(mq2 + 1) * 128,
                              nk * 512:(nk + 1) * 512],
                        ost[:],
                    )

        _ys = {}
        _dn = {}
        _rbp = {}

        def pair_y(b, pr, p):
            g65 = (pr // 2) * 65
            # merged Y chains: rhs/out carry both heads via strided APs
            ys = psY.tile([65, 512], F32, tag="ys", name=f"ys{b}{pr}")
            _ys[(b, pr)] = ys
            p3 = p[:].rearrange("q (h c) -> q h c", h=2)
            y3 = ys[:].rearrange("q (h c) -> q h c", h=2)
            nc.tensor.matmul(ys[:], vs[b][0:SINK, g65:g65 + 65],
                             p3[0:SINK, :, 0:256], start=True, stop=False)
            nc.tensor.matmul(ys[:], vt[(1, b)][:, g65:g65 + 65],
                             p3[:, :, 256:512], start=False, stop=False)
            nc.tensor.matmul(y3[:, :, 0:128], vt[(0, b)][:, g65:g65 + 65],
                             p3[:, :, 768:896], start=False, stop=False,
                             skip_group_check=True)
            nc.tensor.matmul(y3[:, :, 128:256],
                             vt[(3, b)][:, g65:g65 + 65],
                             p3[:, :, 896:1024], start=False, stop=False,
                             skip_group_check=True)
            nc.tensor.matmul(ys[:], vt[(2, b)][:, g65:g65 + 65],
                             p3[:, :, 512:768], start=False, stop=True)

        def pair_dn(b, pr, p):
            # copy the [1,512] denom row (rounds to f32r)
            ys = _ys[(b, pr)]
            dn = spool.tile([65, 512], FR, tag="dn", name=f"dn{b}{pr}")
            _dn[(b, pr)] = dn
            nc.scalar.copy(dn[64:65, :], ys[64:65, :])

        def pair_bcast(b, pr, p):
            # K=1-matmul broadcast across 64 partitions + fast reciprocal
            dn = _dn[(b, pr)]
            rbp = psB.tile([64, 512], F32, tag="blk",
                           name=f"rbp{b}{pr}")
            nc.tensor.matmul(rbp[:], ones[64:65, 0:64], dn[64:65, :],
                             start=True, stop=True)
            rb = spool.tile([64, 512], F32, tag="rb", name=f"rb{b}{pr}")
            nc.vector.reciprocal_approx_fast(rb[:], rbp[:])
            _rbp[(b, pr)] = rb

        def pair_norm(b, pr, p):
            ys = _ys[(b, pr)]
            rb = _rbp[(b, pr)]
            nc.vector.tensor_mul(
                yt[pr][0:64, b * QB:(b + 1) * QB],
                ys[0:64, 0:256], rb[:, 0:256],
            )
            stg = spool.tile([64, QB], BF, tag="stg", name=f"stg{b}{pr}")
            nc.vector.tensor_mul(stg[:], ys[0:64, 256:512],
                                 rb[:, 256:512])
            nc.sync.dma_start(
                yt[pr][64:128, b * QB:(b + 1) * QB], stg[:]
            )
            if b == 0 and pr == 7:
                # batch-0 o_proj overlaps batch-1 attention
                o_proj(0)

        # half-pair software pipeline: the previous pair's Y chain and
        # denominator work are interleaved BETWEEN the current pair's two
        # score halves, so the PE never drains (keeps p-state high) and the
        # ACT queue sees the dn copy before the next exp
        pairs = [(b, pr) for b in range(B) for pr in range(8)]
        st = {}
        for i, (b, pr) in enumerate(pairs):
            p = ppool.tile([128, 2048], BF, tag="p", name=f"p{b}{pr}")
            prev = st.get(i - 1)
            if prev is not None:
                pair_y(*prev)
                pair_dn(*prev)
            scores_half(b, pr, 0, p, 0)        # head 2*pr
            if prev is not None:
                pair_bcast(*prev)
            scores_half(b, pr, 64, p, 1024)    # head 2*pr+1
            if prev is not None:
                pair_norm(*prev)
            st[i] = (b, pr, p)
        last = st[len(pairs) - 1]
        pair_y(*last)
        pair_dn(*last)
        pair_bcast(*last)
        pair_norm(*last)
        o_proj(1)

    nc.compile()
    return nc


# ================================================================ host side
def host_prep(X, Wq, Wk, Wv, Wo):
    """Returns in_maps (list of per-core dicts of numpy arrays)."""
    import ml_dtypes
    bf = np.dtype(ml_dtypes.bfloat16)

    X = np.asarray(X, dtype=np.float32)
    Wq = np.asarray(Wq, dtype=np.float32)
    Wk = np.asarray(Wk, dtype=np.float32)
    Wv = np.asarray(Wv, dtype=np.float32)
    Wo = np.asarray(Wo, dtype=np.float32)

    wq_p = (Wq * np.float32(1.0 / np.sqrt(HD))).astype(bf)
    wq_blob = np.ascontiguousarray(
        wq_p.reshape(8, 128, 1024).transpose(1, 0, 2).reshape(128, 8192))
    wo_blob = np.ascontiguousarray(
        Wo.astype(bf).reshape(8, 128, 1024).transpose(1, 0, 2).reshape(
            128, 8192))
    wkv_blob = np.ascontiguousarray(
        np.concatenate([Wk.astype(bf).reshape(8, 128, 256),
                        Wv.astype(bf).reshape(8, 128, 256)],
                       axis=2).transpose(1, 0, 2).reshape(128, 8 * 512))

    tt = np.arange(T)
    i = tt[:, None]
    j = tt[None, :]
    m_full = (j <= i) & ((j < SINK) | (j >= np.maximum(i - WIN + 1, 0)))
    m_full = m_full.astype(np.float32)

    in_maps = []
    for c in range(NCORES):
        qs = c * QB
        ks = qs - QB  # window start (512 keys ending at qs+256)

        xw = np.zeros((B, KW, D), dtype=np.float32)
        lo = max(ks, 0)
        xw[:, lo - ks:, :] = X[:, lo:ks + KW, :]
        xcat = np.concatenate([xw, X[:, 0:SINK, :]], axis=1)  # [B, KCOL, D]
        xtt = np.ascontiguousarray(
            xcat.transpose(2, 0, 1).reshape(8, 128, B * KCOL)
        ).astype(bf)

        # m_full lookup with out-of-range keys -> 0
        def mf(qrows, krows):
            qrows = np.asarray(qrows)
            krows = np.asarray(krows)
            out = np.zeros((len(qrows), len(krows)), dtype=np.float32)
            val = (krows >= 0) & (krows < T)
            out[:, val] = m_full[np.ix_(qrows, krows[val])]
            return out

        q_all = qs + np.arange(QB)
        # M1: [T(-1) keys qs-128..qs for queries 0:256 | T(0) keys qs..qs+128
        # for queries 0:128], transposed to [key 128, query cols]
        m1 = np.concatenate([
            mf(q_all, ks + 128 + np.arange(128)).T,          # [128, 256]
            mf(qs + np.arange(128), qs + np.arange(128)).T,  # [128, 128]
        ], axis=1)
        # M2: [T(-2) keys qs-256..qs-128 for queries 0:128 | T(1) keys
        # qs+128..qs+256 for queries 128:256]
        m2 = np.concatenate([
            mf(qs + np.arange(128), ks + np.arange(128)).T,
            mf(qs + 128 + np.arange(128), qs + 128 + np.arange(128)).T,
        ], axis=1)
        # sink mask; zero where a window tile serving that query-half
        # already covers key s (T(-2) serves only queries 0:128, T(1) only
        # 128:256, T(-1)/T(0) serve all)
        msk = np.zeros((SINK, QB), dtype=np.float32)
        for s in range(SINK):
            if not (ks <= s < ks + KW):
                msk[s, :] = m_full[qs:qs + QB, s]
            else:
                tk = (s - ks) // 128
                if tk == 0:
                    msk[s, 128:] = m_full[qs + 128:qs + QB, s]
                elif tk == 3:
                    msk[s, :128] = m_full[qs:qs + 128, s]

        mskblob = np.zeros((128, 896), dtype=np.float32)
        mskblob[:, 0:384] = m1
        mskblob[:, 384:640] = m2
        mskblob[0:SINK, 640:896] = msk
        in_maps.append({
            "ONER": np.ones((65, 64), dtype=np.float32),
            "XT": np.ascontiguousarray(
                xtt.transpose(1, 0, 2).reshape(128, 8 * B * KCOL)),
            "WKV": wkv_blob,
            "WQA": wq_blob,
            "WOA": wo_blob,
            "MSK": mskblob.astype(bf),
        })
    return in_maps


_NC_CACHE = {}


def get_nc():
    if "nc" not in _NC_CACHE:
        _NC_CACHE["nc"] = build_nc()
    return _NC_CACHE["nc"]


def kernel(X, Wq, Wk, Wv, Wo):
    in_maps = host_prep(X, Wq, Wk, Wv, Wo)
    nc = get_nc()
    res = run_bass_kernel_spmd(nc, in_maps, list(range(NCORES)))
    out = np.empty((B, T, D), dtype=np.float32)
    for c in range(NCORES):
        out[:, c * QB:(c + 1) * QB, :] = res.results[c]["out"].astype(
            np.float32
        )
    return out


# revision 27
# speedup vs baseline: 1.2039x; 1.2039x over previous
"""AttentionSinkPrefill Trainium2 kernel (8 NeuronCores, sequence-parallel).

Module:   Y = AttnSinkPrefill(X) with sink=4, window=256, causal GQA
          (16 q heads, 4 kv heads, head_dim 64, d_model 1024, B=2, T=2048).

Sharding: sequence-parallel over T.  Core c handles queries
          [256c, 256c+256) for both batches; it needs X rows
          [256c-256, 256c+256) (zero-padded at the left edge) plus the 4
          sink rows, and computes its o_proj output rows completely -- no
          collectives, outputs concatenate on host.  Host prep is layout
          only: bf16 cast, X pre-transposed to X^T d-major tiles, weights
          packed into single DMA blobs, per-core band masks.

Device pipeline (per core, ~137 us vs 301 us baseline):
  1. inputs arrive as 5 blob DMAs (X^T, Wk|Wv, Wq, masks, Wo) -- one
     descriptor-setup each, so the queues ramp immediately
  2. projections on PE (bf16, all N>=256): K^T duplicated onto both
     partition halves (via SBUF->SBUF DMA) so both heads of a same-group
     pair read one K tile; V kept keys-major with a ones column that
     makes the softmax denominator fall out of the Y matmuls
  3. attention per head-pair: 3-key-tile blocking (each 128-query
     half-block touches only the 3 window tiles intersecting its
     sink+window band; triangle masks from host), exp on ACT straight
     from PSUM, mask multiplies on DVE, the pair's two Y chains merged
     into 5 matmuls via [128, 2, 256] strided APs
  4. softmax denominators per pair: ACT row copy (rounds f32r) -> K=1
     matmul broadcast across partitions -> DVE fast reciprocal ->
     normalize (odd head lands via SBUF->SBUF DMA partition shift)
  5. o_proj per batch right after that batch's pairs (overlaps the next
     batch's attention); bf16 output, host casts back to f32
"""

import os
import sys
from contextlib import ExitStack

import numpy as np

# engine-choice switches for bisecting hardware issues
K_SINKMUL = os.environ.get("K_SINKMUL", "dve")   # dve | gpsimd
K_MEMSET = os.environ.get("K_MEMSET", "dve")     # dve | gpsimd

sys.path.insert(0, "/opt/trn_rl_repo")

import concourse.bass as bass
import concourse.bacc as bacc
import concourse.mybir as mybir
import concourse.tile as tile
from concourse.bass_utils import run_bass_kernel_spmd

# ---------------------------------------------------------------- constants
D = 1024          # d_model
NH = 16           # q heads
NKV = 4           # kv heads
HD = 64           # head dim
SINK = 4          # attention sink width
WIN = 256         # sliding window
B = 2
T = 2048
NCORES = 8
QB = T // NCORES  # queries per core = 256
KW = 2 * QB       # window key rows per core = 512
KCOL = KW + SINK  # X^T key columns per batch = 516

F32 = mybir.dt.float32
FR = mybir.dt.float32r
BF = mybir.dt.bfloat16

AF = mybir.ActivationFunctionType


# ================================================================ program
def build_nc():
    nc = bacc.Bacc()

    # inputs packed into few large blobs: descriptor setup for each
    # dma_start serializes on the sync engine, so fewer/bigger transfers
    # start the pipeline sooner
    xt_d = nc.dram_tensor("XT", [128, 8 * B * KCOL], BF, kind="ExternalInput")
    wkv_d = nc.dram_tensor("WKV", [128, 8 * 512], BF, kind="ExternalInput")
    wq_d = nc.dram_tensor("WQA", [128, 8 * 1024], BF, kind="ExternalInput")
    wo_d = nc.dram_tensor("WOA", [128, 8 * 1024], BF, kind="ExternalInput")
    msk_d = nc.dram_tensor("MSK", [128, 896], BF, kind="ExternalInput")
    oner_d = nc.dram_tensor("ONER", [65, 64], FR, kind="ExternalInput")
    out_d = nc.dram_tensor("out", [B, QB, D], BF, kind="ExternalOutput")

    with nc.allow_low_precision(reason="bf16 matmul operands"), \
            tile.TileContext(nc) as tc, ExitStack() as ctx:
        consts = ctx.enter_context(tc.tile_pool(name="consts", bufs=1))
        wpool = ctx.enter_context(tc.tile_pool(name="wpool", bufs=1))
        big = ctx.enter_context(tc.tile_pool(name="big", bufs=1))
        xtp = ctx.enter_context(tc.tile_pool(name="xt", bufs=1))
        qkv = ctx.enter_context(tc.tile_pool(name="qkv", bufs=1))
        ppool = ctx.enter_context(tc.tile_pool(name="pp", bufs=3))
        ypool = ctx.enter_context(tc.tile_pool(name="yp", bufs=1))
        spool = ctx.enter_context(tc.tile_pool(name="sp", bufs=2))
        opool = ctx.enter_context(tc.tile_pool(name="op", bufs=2))
        # PSUM: tag "s" 2x[128,1024]f32 = 4 banks, tag "blk" 2x1 = 2,
        # tag "ys" 2x[65,512] = 2  -> exactly 8 banks
        psS = ctx.enter_context(tc.tile_pool(name="psS", bufs=2, space="PSUM"))
        psB = ctx.enter_context(tc.tile_pool(name="psB", bufs=2, space="PSUM"))
        psY = ctx.enter_context(tc.tile_pool(name="psY", bufs=2, space="PSUM"))

        # -------- DMA: X^T first (projections start on it), then K/V
        # weights (small, consumed first), masks, Q weights; Wo is issued
        # later (only needed at o_proj)
        xall = xtp.tile([128, 8 * B * KCOL], BF, tag="xall", name="xall")
        nc.sync.dma_start(xall[:], xt_d[:])
        xt = [xall[:, d * B * KCOL:(d + 1) * B * KCOL] for d in range(8)]

        wkv = wpool.tile([128, 8 * 512], BF, tag="wkv", name="wkv")
        nc.sync.dma_start(wkv[:], wkv_d[:])
        wk = [wkv[:, d * 512:d * 512 + 256] for d in range(8)]
        wv = [wkv[:, d * 512 + 256:(d + 1) * 512] for d in range(8)]

        mall = consts.tile([128, 896], BF, tag="mall")
        nc.sync.dma_start(mall[:], msk_d[:])
        m1 = mall[:, 0:384]
        m2 = mall[:, 384:640]
        ms = mall[0:SINK, 640:896]
        ones = consts.tile([65, 64], FR, tag="ones")
        nc.sync.dma_start(ones[:], oner_d[:])

        wqa = wpool.tile([128, 8 * 1024], BF, tag="wqa", name="wqa")
        nc.sync.dma_start(wqa[:], wq_d[:])
        wq = [wqa[:, d * 1024:(d + 1) * 1024] for d in range(8)]

        mseng = nc.gpsimd if K_MEMSET == "gpsimd" else nc.vector
        # -------- persistent per-core tensors
        qt = [qkv.tile([128, B * QB], BF, tag=f"qt{m}", name=f"qt{m}")
              for m in range(8)]
        # K^T per kv head, duplicated onto both partition halves so both
        # heads of a same-group pair can use their own 64-partition slice
        ktd = [qkv.tile([128, B * KW], BF, tag=f"ktd{g}", name=f"ktd{g}")
               for g in range(NKV)]
        # zero-padded duplicated sink K^T: [128, key 0:4 real | 4:128 zero]
        ktp = {}
        for g in range(NKV):
            for b in range(B):
                tl = qkv.tile([128, 128], BF, tag=f"ktp{g}{b}", name=f"ktp{g}{b}")
                mseng.memset(tl[:], 0.0)
                ktp[(g, b)] = tl
        # V keys-major with a ones column per kv head (softmax denominator)
        vt = {}
        for tki in range(4):
            for b in range(B):
                tl = qkv.tile([128, NKV * 65], BF,
                              tag=f"vt{tki}{b}", name=f"vt{tki}{b}")
                mseng.memset(tl[:, 64:NKV * 65:65], 1.0)
                vt[(tki, b)] = tl
        vs = {}
        for b in range(B):
            tl = qkv.tile([SINK, NKV * 65], BF, tag=f"vs{b}", name=f"vs{b}")
            mseng.memset(tl[0:SINK, 64:NKV * 65:65], 1.0)
            vs[b] = tl
        yt = [ypool.tile([128, B * QB], BF, tag=f"yt{m}", name=f"yt{m}")
              for m in range(8)]

        # ---------------- stage 2: projections.  Emission order matters
        # (engine queues execute in order): K(b0), V(b0) first -- their
        # weights are small and land early while X^T still streams -- then
        # Q (bigger Wq lands meanwhile), then K(b1), V(b1).
        def q_proj():
            for m in range(8):
                ps = psB.tile([128, 512], F32, tag="blk", name=f"qps{m}")
                for d in range(8):
                    rhs = xt[d].rearrange(
                        "p (b c) -> p b c", b=B
                    )[:, :, KW - QB:KW]
                    nc.tensor.matmul(
                        ps[:],
                        wq[d][:, m * 128:(m + 1) * 128],
                        rhs,
                        start=(d == 0), stop=(d == 7),
                    )
                nc.scalar.copy(qt[m][:], ps[:])

        def kv_proj(b):
            for m in range(2):
                gA, gB = 2 * m, 2 * m + 1
                ps = psB.tile([128, 512], F32, tag="blk", name=f"kps{b}{m}")
                for d in range(8):
                    nc.tensor.matmul(
                        ps[:],
                        wk[d][:, m * 128:(m + 1) * 128],
                        xt[d][:, b * KCOL:b * KCOL + KW],
                        start=(d == 0), stop=(d == 7),
                    )
                # top half of each dup tile via engine copy (no partition
                # shift), the other half via SBUF->SBUF DMA duplication
                nc.vector.tensor_copy(
                    ktd[gA][0:64, b * KW:(b + 1) * KW], ps[0:64, :])
                nc.vector.tensor_copy(
                    ktd[gB][64:128, b * KW:(b + 1) * KW], ps[64:128, :])
                nc.sync.dma_start(
                    ktd[gA][64:128, b * KW:(b + 1) * KW],
                    ktd[gA][0:64, b * KW:(b + 1) * KW])
                nc.sync.dma_start(
                    ktd[gB][0:64, b * KW:(b + 1) * KW],
                    ktd[gB][64:128, b * KW:(b + 1) * KW])

                ps2 = psB.tile([128, 512], F32, tag="blk", name=f"ksps{b}{m}")
                for d in range(8):
                    nc.tensor.matmul(
                        ps2[:, 0:SINK],
                        wk[d][:, m * 128:(m + 1) * 128],
                        xt[d][:, b * KCOL + KW:b * KCOL + KCOL],
                        start=(d == 0), stop=(d == 7),
                    )
                nc.vector.tensor_copy(
                    ktp[(gA, b)][0:64, 0:SINK], ps2[0:64, 0:SINK])
                nc.vector.tensor_copy(
                    ktp[(gB, b)][64:128, 0:SINK], ps2[64:128, 0:SINK])
                nc.sync.dma_start(
                    ktp[(gA, b)][64:128, 0:SINK], ktp[(gA, b)][0:64, 0:SINK])
                nc.sync.dma_start(
                    ktp[(gB, b)][0:64, 0:SINK], ktp[(gB, b)][64:128, 0:SINK])

            for tki in range(4):
                ps = psB.tile([128, 512], F32, tag="blk", name=f"vps{b}{tki}")
                for d in range(8):
                    nc.tensor.matmul(
                        ps[:, 0:NKV * HD],
                        xt[d][:, b * KCOL + tki * 128:b * KCOL + (tki + 1) * 128],
                        wv[d][:],
                        start=(d == 0), stop=(d == 7),
                    )
                nc.vector.tensor_copy(
                    vt[(tki, b)][:].rearrange("p (g c) -> p g c", c=65)[:, :, 0:HD],
                    ps[:, 0:NKV * HD].rearrange("p (g c) -> p g c", c=HD),
                )
            ps = psB.tile([128, 512], F32, tag="blk", name=f"vsps{b}")
            for d in range(8):
                nc.tensor.matmul(
                    ps[0:SINK, 0:NKV * HD],
                    xt[d][:, b * KCOL + KW:b * KCOL + KCOL],
                    wv[d][:],
                    start=(d == 0), stop=(d == 7),
                )
            nc.vector.tensor_copy(
                vs[b][0:SINK, :].rearrange("p (g c) -> p g c", c=65)[:, :, 0:HD],
                ps[0:SINK, 0:NKV * HD].rearrange("p (g c) -> p g c", c=HD),
            )

        kv_proj(0)
        kv_proj(1)
        q_proj()

        # Wo DMA now: streams during attention, consumed by o_proj
        woa = big.tile([128, 8 * 1024], BF, tag="woa", name="woa")
        nc.sync.dma_start(woa[:], wo_d[:])
        wo = [woa[:, m * 1024:(m + 1) * 1024] for m in range(8)]

        # ---------------- stage 3: attention
        # p layout per head (1024 bf16 cols inside the pair tile):
        #   [0:256]    sink scores (keys 0:4 real via zero-padded ktp)
        #   [256:512]  key tile T(-1) = window cols 128:256, queries 0:256
        #   [512:768]  key tile T(0)  = window cols 256:384, queries 0:256
        #   [768:896]  key tile T(-2) = window cols 0:128,   queries 0:128
        #   [896:1024] key tile T(1)  = window cols 384:512, queries 128:256
        def scores_half(b, pr, kb, p, off):
            """scores + exp + masks for one head (partition base kb)."""
            g = pr // 2
            qall = qt[pr][kb:kb + 64, b * QB:(b + 1) * QB]
            qhb0 = qt[pr][kb:kb + 64, b * QB:b * QB + 128]
            qhb1 = qt[pr][kb:kb + 64, b * QB + 128:(b + 1) * QB]
            kw0 = b * KW

            sp = psS.tile([128, 1024], F32, tag="s", name=f"s{b}{pr}{kb}")
            nc.tensor.matmul(sp[:, 0:256], ktp[(g, b)][kb:kb + 64, :],
                             qall, start=True, stop=True)
            nc.tensor.matmul(sp[:, 256:512],
                             ktd[g][kb:kb + 64, kw0 + 128:kw0 + 256],
                             qall, start=True, stop=True)
            nc.tensor.matmul(sp[:, 512:768],
                             ktd[g][kb:kb + 64, kw0 + 256:kw0 + 384],
                             qall, start=True, stop=True)
            nc.tensor.matmul(sp[:, 768:896],
                             ktd[g][kb:kb + 64, kw0 + 0:kw0 + 128],
                             qhb0, start=True, stop=True)
            nc.tensor.matmul(sp[:, 896:1024],
                             ktd[g][kb:kb + 64, kw0 + 384:kw0 + 512],
                             qhb1, start=True, stop=True)

            nc.scalar.activation(p[:, off:off + 1024], sp[:], AF.Exp)
            nc.vector.tensor_mul(p[:, off + 256:off + 640],
                                 p[:, off + 256:off + 640], m1)
            nc.vector.tensor_mul(p[:, off + 768:off + 1024],
                                 p[:, off + 768:off + 1024], m2)
            smeng = nc.gpsimd if K_SINKMUL == "gpsimd" else nc.vector
            smeng.tensor_mul(p[0:SINK, off:off + 256],
                             p[0:SINK, off:off + 256], ms)

        def o_proj(b):
            for mq2 in range(2):
                for nk in range(2):
                    po = psB.tile([128, 512], F32, tag="blk",
                                  name=f"po{b}{mq2}{nk}")
                    for m in range(8):
                        nc.tensor.matmul(
                            po[:],
                            yt[m][:, b * QB + mq2 * 128:b * QB + (mq2 + 1) * 128],
                            wo[m][:, nk * 512:(nk + 1) * 512],
                            start=(m == 0), stop=(m == 7),
                        )
                    ost = opool.tile([128, 512], BF, tag="ost",
                                     name=f"o{b}{mq2}{nk}")
                    nc.vector.tensor_copy(ost[:], po[:])
                    nc.sync.dma_start(
                        out_d[b, mq2 * 128:(mq2 + 1) * 128,
                              nk * 512:(nk + 1) * 512],
                        ost[:],
                    )

        _ys = {}
        _dn = {}
        _rbp = {}

        def pair_y(b, pr, p):
            g65 = (pr // 2) * 65
            # merged Y chains: rhs/out carry both heads via strided APs
            ys = psY.tile([65, 512], F32, tag="ys", name=f"ys{b}{pr}")
            _ys[(b, pr)] = ys
            p3 = p[:].rearrange("q (h c) -> q h c", h=2)
            y3 = ys[:].rearrange("q (h c) -> q h c", h=2)
            nc.tensor.matmul(ys[:], vs[b][0:SINK, g65:g65 + 65],
                             p3[0:SINK, :, 0:256], start=True, stop=False)
            nc.tensor.matmul(ys[:], vt[(1, b)][:, g65:g65 + 65],
                             p3[:, :, 256:512], start=False, stop=False)
            nc.tensor.matmul(y3[:, :, 0:128], vt[(0, b)][:, g65:g65 + 65],
                             p3[:, :, 768:896], start=False, stop=False,
                             skip_group_check=True)
            nc.tensor.matmul(y3[:, :, 128:256],
                             vt[(3, b)][:, g65:g65 + 65],
                             p3[:, :, 896:1024], start=False, stop=False,
                             skip_group_check=True)
            nc.tensor.matmul(ys[:], vt[(2, b)][:, g65:g65 + 65],
                             p3[:, :, 512:768], start=False, stop=True)

        def pair_dn(b, pr, p):
            # copy the [1,512] denom row (rounds to f32r)
            ys = _ys[(b, pr)]
            dn = spool.tile([65, 512], FR, tag="dn", name=f"dn{b}{pr}")
            _dn[(b, pr)] = dn
            nc.scalar.copy(dn[64:65, :], ys[64:65, :])

        def pair_bcast(b, pr, p):
            # K=1-matmul broadcast across 64 partitions + fast reciprocal
            dn = _dn[(b, pr)]
            rbp = psB.tile([64, 512], F32, tag="blk",
                           name=f"rbp{b}{pr}")
            nc.tensor.matmul(rbp[:], ones[64:65, 0:64], dn[64:65, :],
                             start=True, stop=True)
            rb = spool.tile([64, 512], F32, tag="rb", name=f"rb{b}{pr}")
            nc.vector.reciprocal_approx_fast(rb[:], rbp[:])
            _rbp[(b, pr)] = rb

        def pair_norm(b, pr, p):
            ys = _ys[(b, pr)]
            rb = _rbp[(b, pr)]
            nc.vector.tensor_mul(
                yt[pr][0:64, b * QB:(b + 1) * QB],
                ys[0:64, 0:256], rb[:, 0:256],
            )
            stg = spool.tile([64, QB], BF, tag="stg", name=f"stg{b}{pr}")
            nc.vector.tensor_mul(stg[:], ys[0:64, 256:512],
                                 rb[:, 256:512])
            nc.sync.dma_start(
                yt[pr][64:128, b * QB:(b + 1) * QB], stg[:]
            )
            if b == 0 and pr == 7:
                # batch-0 o_proj overlaps batch-1 attention
                o_proj(0)

        # half-pair software pipeline: the previous pair's Y chain and
        # denominator work are interleaved BETWEEN the current pair's two
        # score halves, so the PE never drains (keeps p-state high) and the
        # ACT queue sees the dn copy before the next exp
        pairs = [(b, pr) for b in range(B) for pr in range(8)]
        st = {}
        for i, (b, pr) in enumerate(pairs):
            p = ppool.tile([128, 2048], BF, tag="p", name=f"p{b}{pr}")
            prev = st.get(i - 1)
            if prev is not None:
                pair_y(*prev)
                pair_dn(*prev)
            scores_half(b, pr, 0, p, 0)        # head 2*pr
            if prev is not None:
                pair_bcast(*prev)
            scores_half(b, pr, 64, p, 1024)    # head 2*pr+1
            if prev is not None:
                pair_norm(*prev)
            st[i] = (b, pr, p)
        last = st[len(pairs) - 1]
        pair_y(*last)
        pair_dn(*last)
        pair_bcast(*last)
        pair_norm(*last)
        o_proj(1)

    nc.compile()
    return nc


# ================================================================ host side
def host_prep(X, Wq, Wk, Wv, Wo):
    """Returns in_maps (list of per-core dicts of numpy arrays)."""
    import ml_dtypes
    bf = np.dtype(ml_dtypes.bfloat16)

    X = np.asarray(X, dtype=np.float32)
    Wq = np.asarray(Wq, dtype=np.float32)
    Wk = np.asarray(Wk, dtype=np.float32)
    Wv = np.asarray(Wv, dtype=np.float32)
    Wo = np.asarray(Wo, dtype=np.float32)

    wq_p = (Wq * np.float32(1.0 / np.sqrt(HD))).astype(bf)
    wq_blob = np.ascontiguousarray(
        wq_p.reshape(8, 128, 1024).transpose(1, 0, 2).reshape(128, 8192))
    wo_blob = np.ascontiguousarray(
        Wo.astype(bf).reshape(8, 128, 1024).transpose(1, 0, 2).reshape(
            128, 8192))
    wkv_blob = np.ascontiguousarray(
        np.concatenate([Wk.astype(bf).reshape(8, 128, 256),
                        Wv.astype(bf).reshape(8, 128, 256)],
                       axis=2).transpose(1, 0, 2).reshape(128, 8 * 512))

    tt = np.arange(T)
    i = tt[:, None]
    j = tt[None, :]
    m_full = (j <= i) & ((j < SINK) | (j >= np.maximum(i - WIN + 1, 0)))
    m_full = m_full.astype(np.float32)

    in_maps = []
    for c in range(NCORES):
        qs = c * QB
        ks = qs - QB  # window start (512 keys ending at qs+256)

        xw = np.zeros((B, KW, D), dtype=np.float32)
        lo = max(ks, 0)
        xw[:, lo - ks:, :] = X[:, lo:ks + KW, :]
        xcat = np.concatenate([xw, X[:, 0:SINK, :]], axis=1)  # [B, KCOL, D]
        xtt = np.ascontiguousarray(
            xcat.transpose(2, 0, 1).reshape(8, 128, B * KCOL)
        ).astype(bf)

        # m_full lookup with out-of-range keys -> 0
        def mf(qrows, krows):
            qrows = np.asarray(qrows)
            krows = np.asarray(krows)
            out = np.zeros((len(qrows), len(krows)), dtype=np.float32)
            val = (krows >= 0) & (krows < T)
            out[:, val] = m_full[np.ix_(qrows, krows[val])]
            return out

        q_all = qs + np.arange(QB)
        # M1: [T(-1) keys qs-128..qs for queries 0:256 | T(0) keys qs..qs+128
        # for queries 0:128], transposed to [key 128, query cols]
        m1 = np.concatenate([
            mf(q_all, ks + 128 + np.arange(128)).T,          # [128, 256]
            mf(qs + np.arange(128), qs + np.arange(128)).T,  # [128, 128]
        ], axis=1)
        # M2: [T(-2) keys qs-256..qs-128 for queries 0:128 | T(1) keys
        # qs+128..qs+256 for queries 128:256]
        m2 = np.concatenate([
            mf(qs + np.arange(128), ks + np.arange(128)).T,
            mf(qs + 128 + np.arange(128), qs + 128 + np.arange(128)).T,
        ], axis=1)
        # sink mask; zero where a window tile serving that query-half
        # already covers key s (T(-2) serves only queries 0:128, T(1) only
        # 128:256, T(-1)/T(0) serve all)
        msk = np.zeros((SINK, QB), dtype=np.float32)
        for s in range(SINK):
            if not (ks <= s < ks + KW):
                msk[s, :] = m_full[qs:qs + QB, s]
            else:
                tk = (s - ks) // 128
                if tk == 0:
                    msk[s, 128:] = m_full[qs + 128:qs + QB, s]
                elif tk == 3:
                    msk[s, :128] = m_full[qs:qs + 128, s]

        mskblob = np.zeros((128, 896), dtype=np.float32)
        mskblob[:, 0:384] = m1
        mskblob[:, 384:640] = m2
        mskblob[0:SINK, 640:896] = msk
        in_maps.append({
            "ONER": np.ones((65, 64), dtype=np.float32),
            "XT": np.ascontiguousarray(
                xtt.transpose(1, 0, 2).reshape(128, 8 * B * KCOL)),
            "WKV": wkv_blob,
            "WQA": wq_blob,
            "WOA": wo_blob,
            "MSK": mskblob.astype(bf),
        })
    return in_maps


_NC_CACHE = {}


def get_nc():
    if "nc" not in _NC_CACHE:
        _NC_CACHE["nc"] = build_nc()
    return _NC_CACHE["nc"]


def kernel(X, Wq, Wk, Wv, Wo):
    in_maps = host_prep(X, Wq, Wk, Wv, Wo)
    nc = get_nc()
    res = run_bass_kernel_spmd(nc, in_maps, list(range(NCORES)))
    out = np.empty((B, T, D), dtype=np.float32)
    for c in range(NCORES):
        out[:, c * QB:(c + 1) * QB, :] = res.results[c]["out"].astype(
            np.float32
        )
    return out
